# revision 8
# baseline (speedup 1.0000x reference)
"""IRevNetSqueeze (pixel-unshuffle, block=2) Trainium2 Bass kernel.

out[b, 4c + 2i + j, ho, wo] = x[b, c, 2*ho + i, 2*wo + j]

Full input x: (16, 16, 512, 512) f32 -> output (16, 64, 256, 256) f32.

Sharding: pure data parallelism over the batch dim — core k handles
batches [2k, 2k+2). No cross-core communication.

Per-core dataflow (8 iterations over (b, cg, hh)):
  1. HWDGE load [128(p=ho), CG(c), 2(i), 512(w)] f32 = 4 MiB.
     DRAM rows h=2p and h=2p+1 are adjacent, so (i, w) merges into
     4 KiB contiguous descriptors.
  2. One strided DVE copy de-interleaves columns:
     S[p, (2c+i), j, wo] = L[p, (2c+i), 2*wo + j].
  3. HWDGE store [128(p), 32(chl), 256(wo)] = 4 MiB, 1 KiB descriptors
     (output rows are contiguous in DRAM).

Buffering: one pool with 4 fixed single-slot buffers (tags T0-T3); load t
fills buffer t%4, the shuffle output S_t uses buffer (t+2)%4, giving
double-buffered loads/stores with alternating occupants.
"""

import numpy as np

import concourse.bass as bass
import concourse.tile as tile
from concourse import bacc, mybir
from concourse.bass_utils import run_bass_kernel_spmd

B, C, H, W = 16, 16, 512, 512
N_CORES = 8
BPC = B // N_CORES  # batches per core = 2
HO, WO = H // 2, W // 2  # 256, 256
CG = 4  # input channels per tile group
P = 128  # SBUF partitions
NBUF = 6  # fixed single-slot buffers in the rotation

_cached_nc = None


def _build_nc() -> bass.Bass:
    nc = bacc.Bacc("TRN2", target_bir_lowering=False, debug=False,
                   num_devices=N_CORES)
    x = nc.dram_tensor("x", [BPC, C, H, W], mybir.dt.float32,
                       kind="ExternalInput").ap()
    y = nc.dram_tensor("y", [BPC, 4 * C, HO, WO], mybir.dt.float32,
                       kind="ExternalOutput").ap()

    n_cg = C // CG  # 2
    n_hh = HO // P  # 2

    with tile.TileContext(nc) as tc:
        with tc.tile_pool(name="buf", bufs=1) as pool:
            t = 0
            for b in range(BPC):
                # h = 256*hh + 2*p + i
                xv = x[b].rearrange("c (hh p i) w -> hh p c i w", hh=n_hh, i=2)
                # ch = 32*cg + chl, ho = 128*hh + p
                yv = y[b].rearrange("(cg chl) (hh p) wo -> cg hh p chl wo",
                                    cg=n_cg, hh=n_hh)
                for cg in range(n_cg):
                    for hh in range(n_hh):
                        L = pool.tile([P, CG, 2, W], mybir.dt.float32,
                                      tag=f"T{t % NBUF}", name=f"L{t}")
                        # Loads on the SP HWDGE ring.
                        nc.sync.dma_start(
                            L[:], xv[hh, :, cg * CG:(cg + 1) * CG, :, :])

                        S = pool.tile([P, 4 * CG, WO], mybir.dt.float32,
                                      tag=f"T{(t + NBUF // 2) % NBUF}",
                                      name=f"S{t}")
                        # k = 2c + i (uniform stride in both L and S)
                        in_ap = L.rearrange("p c i (wo j) -> p (c i) j wo", j=2)
                        out_ap = S.rearrange("p (k j) wo -> p k j wo", j=2)
                        nc.vector.tensor_copy(out_ap, in_ap)
                        t += 1

                        # Stores on the ACT HWDGE ring so store sem-waits
                        # never block load issue.
                        nc.scalar.dma_start(yv[cg, hh], S[:])
    nc.compile()
    return nc


def _get_nc() -> bass.Bass:
    global _cached_nc
    if _cached_nc is None:
        _cached_nc = _build_nc()
    return _cached_nc


def _run(x: np.ndarray, **kwargs):
    """Shard, run on 8 cores, gather. Returns (y_full, BassKernelResults)."""
    x = np.ascontiguousarray(x, dtype=np.float32)
    assert x.shape == (B, C, H, W)
    nc = _get_nc()
    in_maps = [{"x": x[k * BPC:(k + 1) * BPC]} for k in range(N_CORES)]
    res = run_bass_kernel_spmd(nc, in_maps, core_ids=list(range(N_CORES)),
                               **kwargs)
    y = np.concatenate([r["y"] for r in res.results], axis=0)
    return y, res


def kernel(x: np.ndarray) -> np.ndarray:
    y, _ = _run(x)
    return y
